# revision 4
# baseline (speedup 1.0000x reference)
"""EuclideanVisitEncoder Trainium2 kernel — SWDGE dma_gather version.

Masked-mean embedding lookup:
    out[n, :] = mean over c of emb[code_ids[n, c]] where code_ids[n, c] >= 0

Strategy (8 NeuronCores, SPMD data-parallel over visits):
  - Shard the 16384 visits into 8 x 2048.  Per core, visits map to
    (group g = v // 128, partition p = v % 128), 16 groups.
  - The table is reorganized on the host into 25001 super-rows x 4
    sub-rows of 256B.  dma_gather indexes super-rows with int16 (< 32768),
    and 4 "bucket" gathers with DRAM base offsets k*256B fetch sub-row k.
    Each descriptor moves exactly one 256B embedding row; one dma_gather
    instruction carries thousands of descriptors spread over all 16 DMA
    engines (vs. one 128-descriptor indirect DMA per code column before).
  - The host assigns each unique id a (super, sub) slot, balancing the
    per-(visit, sub) counts so every visit needs at most C slots per
    bucket; unused slots point at an all-zero super-row and contribute 0.
  - Per visit-quarter (4 groups), 4 bucket gathers land in one SBUF tile
    [128, 4, 4*C, 64]; DVE reduces over the C axis per bucket, adds the
    4 bucket partials, and scales by host-computed 1/max(count,1).
  - Gather idx arrays are int16, wrapped in 16 partitions (idx i at
    partition i%16, column i//16) and replicated across the 8 Q7 cores
    (partition groups 16j..16j+15) — the ucode requires the replication.
No cross-core communication is needed.
"""

import sys

for _p in ("/opt/trn_rl_repo",):
    if _p not in sys.path:
        sys.path.append(_p)

import numpy as np

import concourse.bass as bass
import concourse.mybir as mybir
from concourse.bass_utils import run_bass_kernel_spmd
from concourse.library_config import mlp
from concourse.library_overlay import lower_extended_insts

NUM_CODES = 100000
DIM = 64
N_VISITS = 16384
MAX_CODES = 64
N_CORES = 8
VPC = N_VISITS // N_CORES  # visits per core
P = 128
N_GROUPS = VPC // P        # 16
NSUP = NUM_CODES // 4 + 1  # 25001 super-rows; the last is all zero
ZSUP = NSUP - 1
C = 24                     # max slots per (visit, bucket); 3 chunks of 8
CCH = 8                    # idx columns per chunk
NCH = C // CCH             # chunks per (group, bucket)
NIQ = P * CCH              # 1024 idxs per gather instruction (HW cap)
ICOLS = NIQ // 16          # idx columns per instruction
NINST = 4 * NCH            # gather instructions per group


def build_bass(reps: int = 1):
    nc = bass.Bass()
    table = nc.declare_dram_parameter("table", [NSUP, 4 * DIM], mybir.dt.float32, isOutput=False)
    idxs = nc.declare_dram_parameter("idxs", [128, 16 * NINST * ICOLS], mybir.dt.int16, isOutput=False)
    recip = nc.declare_dram_parameter("recip", [128, N_GROUPS], mybir.dt.float32, isOutput=False)
    out = nc.declare_dram_parameter("out", [VPC, DIM], mybir.dt.float32, isOutput=True)

    with (
        nc.sbuf_tensor("idx_sb", [128, 16 * NINST * ICOLS], mybir.dt.int16) as idx_sb,
        nc.sbuf_tensor("recip_sb", [128, N_GROUPS], mybir.dt.float32) as recip_sb,
        nc.sbuf_tensor("dst0", [128, 4 * C * DIM], mybir.dt.float32) as dst0,
        nc.sbuf_tensor("dst1", [128, 4 * C * DIM], mybir.dt.float32) as dst1,
        nc.sbuf_tensor("red", [128, 4 * DIM], mybir.dt.float32) as red,
        nc.sbuf_tensor("out_all", [128, N_GROUPS * DIM], mybir.dt.float32) as out_all,
        nc.semaphore("s_io") as s_io,
        nc.semaphore("s_g") as s_g,
        nc.semaphore("s_r") as s_r,
        nc.semaphore("s_d") as s_d,
        nc.semaphore("s_out") as s_out,
    ):
        dst = [dst0, dst1]
        nc.gpsimd.load_library(mlp)
        niq_reg = nc.gpsimd.to_reg(NIQ)
        nc.gpsimd.dma_start(idx_sb[:, :], idxs[:, :]).then_inc(s_io, 16)
        nc.gpsimd.dma_start(recip_sb[:, :], recip[:, :]).then_inc(s_io, 16)
        nc.gpsimd.wait_ge(s_io, 32)

        g_cnt = 0   # cumulative gather sem target
        r_cnt = 0   # cumulative group-reduce count
        d_cnt = 0   # cumulative done-scale count
        o_cnt = 0   # cumulative out-store count
        for rep in range(reps):
            for g in range(N_GROUPS):
                ga = rep * N_GROUPS + g  # absolute group index
                buf = dst[ga % 2]
                if ga >= 2:
                    # WAR: this buffer's previous reduces must be done
                    nc.gpsimd.wait_ge(s_r, ga - 1)
                for k in range(4):
                    for ch in range(NCH):
                        t = (g * 4 + k) * NCH + ch
                        base = (k * C + ch * CCH) * DIM
                        nc.gpsimd.dma_gather(
                            buf[:, base : base + CCH * DIM].rearrange(
                                "p (c d) -> p c d", d=DIM
                            ),
                            table[:, k * DIM : (k + 1) * DIM],
                            idx_sb[:, t * ICOLS : (t + 1) * ICOLS],
                            NIQ,
                            niq_reg,
                            DIM,
                            elem_step=4 * DIM,
                        ).then_inc(s_g, 16)
                g_cnt += 16 * NINST
                nc.vector.wait_ge(s_g, g_cnt)
                for k in range(4):
                    ri = nc.vector.reduce_sum(
                        out=red[:, k * DIM : (k + 1) * DIM],
                        in_=buf[:, k * C * DIM : (k + 1) * C * DIM].rearrange(
                            "p (c d) -> p d c", d=DIM
                        ),
                        axis=mybir.AxisListType.X,
                    )
                    if k == 3:
                        r_cnt += 1
                        ri.then_inc(s_r, 1)
                nc.vector.tensor_add(red[:, 0:DIM], red[:, 0:DIM], red[:, DIM : 2 * DIM])
                nc.vector.tensor_add(
                    red[:, 2 * DIM : 3 * DIM], red[:, 2 * DIM : 3 * DIM], red[:, 3 * DIM : 4 * DIM]
                )
                nc.vector.tensor_add(red[:, 0:DIM], red[:, 0:DIM], red[:, 2 * DIM : 3 * DIM])
                si = nc.vector.tensor_scalar_mul(
                    out_all[:, g * DIM : (g + 1) * DIM],
                    red[:, 0:DIM],
                    recip_sb[:, g : g + 1],
                )
                if g == N_GROUPS - 1:
                    d_cnt += 1
                    si.then_inc(s_d, 1)
            nc.gpsimd.wait_ge(s_d, d_cnt)
            nc.gpsimd.dma_start(
                out.rearrange("(g p) d -> p g d", p=P),
                out_all[:, :].rearrange("p (g d) -> p g d", d=DIM),
            ).then_inc(s_out, 16)
            o_cnt += 16
        nc.gpsimd.wait_ge(s_out, o_cnt)
    lower_extended_insts(nc)
    return nc


def _host_prep(code_ids: np.ndarray, emb_weight: np.ndarray, n_cores: int = N_CORES) -> list:
    """Per-core input maps: permuted table, gather idx arrays, recips."""
    emb = np.ascontiguousarray(emb_weight.astype(np.float32))
    in_maps = []
    for core in range(n_cores):
        ids = np.asarray(code_ids[core * VPC : (core + 1) * VPC]).astype(np.int64)
        valid = ids >= 0
        counts = valid.sum(1)
        recip_arr = np.ascontiguousarray(
            (1.0 / np.maximum(counts, 1.0)).astype(np.float32).reshape(N_GROUPS, P).T
        )

        vis, _cc = np.nonzero(valid)
        lid = ids[valid]  # id per valid lookup
        order = np.argsort(lid, kind="stable")
        lid_s, vis_s = lid[order], vis[order]
        uniq, starts = np.unique(lid_s, return_index=True)
        ends = np.append(starts[1:], len(lid_s))
        mult = ends - starts

        L = np.zeros((VPC, 4), np.int32)
        k_lookup = np.empty(len(lid_s), np.int8)
        col_lookup = np.empty(len(lid_s), np.int32)
        k_of_u = np.empty(len(uniq), np.int8)
        s_of_u = np.empty(len(uniq), np.int32)
        nsl = [0, 0, 0, 0]

        # multis first (greedy: min worst-case load), then singles water-fill
        m_order = np.argsort(-mult, kind="stable")
        for ui in m_order:
            if mult[ui] < 2:
                break
            cs = vis_s[starts[ui] : ends[ui]]
            cu, cm = np.unique(cs, return_counts=True)
            best_k, best_key = 0, None
            for k in range(4):
                key = ((L[cu, k] + cm).max(), L[cu, k].sum())
                if best_key is None or key < best_key:
                    best_key, best_k = key, k
            k_of_u[ui] = best_k
            s_of_u[ui] = nsl[best_k]
            nsl[best_k] += 1
            for j in range(starts[ui], ends[ui]):
                v = vis_s[j]
                k_lookup[j] = best_k
                col_lookup[j] = L[v, best_k]
                L[v, best_k] += 1
        # singles: per-visit water fill
        s_mask = mult == 1
        s_ui = np.flatnonzero(s_mask)
        s_pos = starts[s_ui]  # lookup position of each single
        s_vis = vis_s[s_pos]
        vorder = np.argsort(s_vis, kind="stable")
        for j in vorder:
            ui, pos, v = s_ui[j], s_pos[j], s_vis[j]
            k = int(np.argmin(L[v]))
            k_of_u[ui] = k
            s_of_u[ui] = nsl[k]
            nsl[k] += 1
            k_lookup[pos] = k
            col_lookup[pos] = L[v, k]
            L[v, k] += 1

        mx = L.max()
        assert mx <= C, f"core {core}: max bucket load {mx} > C={C}"
        assert max(nsl) <= ZSUP, nsl

        tab = np.zeros((NSUP, 4, DIM), np.float32)
        tab[s_of_u, k_of_u] = emb[uniq]

        # idx array: instruction t = (g*4 + k)*NCH + col//CCH ; i = (col%CCH)*128 + p
        idx_arr = np.full((16, 16 * NINST * ICOLS), ZSUP, np.int16)
        g_l = vis_s // P
        p_l = vis_s % P
        t_l = (g_l * 4 + k_lookup) * NCH + col_lookup // CCH
        i_l = (col_lookup % CCH) * P + p_l
        s_l = s_of_u[np.searchsorted(uniq, lid_s)]
        idx_arr[i_l % 16, t_l * ICOLS + i_l // 16] = s_l.astype(np.int16)
        idx_arr = np.tile(idx_arr, (8, 1))  # replicate across the 8 Q7 cores

        in_maps.append(
            {
                "table": np.ascontiguousarray(tab.reshape(NSUP, 4 * DIM)),
                "idxs": np.ascontiguousarray(idx_arr),
                "recip": recip_arr,
            }
        )
    return in_maps


_NC_CACHE = {}


def _get_nc(reps: int = 1):
    if reps not in _NC_CACHE:
        _NC_CACHE[reps] = build_bass(reps)
    return _NC_CACHE[reps]


def core0_inputs(code_ids: np.ndarray, emb_weight: np.ndarray) -> dict:
    """Core 0's input map (used by test.py's timing path)."""
    return _host_prep(np.asarray(code_ids)[:VPC], emb_weight, n_cores=1)[0]


def kernel(code_ids: np.ndarray, emb_weight: np.ndarray) -> np.ndarray:
    assert code_ids.shape == (N_VISITS, MAX_CODES)
    assert emb_weight.shape == (NUM_CODES, DIM)
    in_maps = _host_prep(code_ids, emb_weight)
    nc = _get_nc(1)
    res = run_bass_kernel_spmd(nc, in_maps, list(range(N_CORES)))
    return np.concatenate([res.results[k]["out"] for k in range(N_CORES)], axis=0)
